# revision 1
# baseline (speedup 1.0000x reference)
"""BiLSTM Trainium2 kernel: B=64, T=512, D=256, H=256, 8 NeuronCores.

Sharding: batch 8-way (8 sequences per core). Each core runs BOTH
directions (forward + backward) as two independent recurrent chains so
the engines can interleave them (one chain's elementwise hides under the
other chain's matmuls).

Host (numpy) does all data movement that is pure layout: per-length
sequence reversal for the backward direction, transposes into the
[feature-on-partition] layouts the device wants, bias folding, g-row x2
prescale (tanh(g) = 2*sigmoid(2g)-1 rides in the wide sigmoid), and the
final gather/mask/concat.

Device (per core):
  phase 1: x projection  xpT[g,(t,b)] = WihT.T @ xT, paced into the
      recurrence; psum->SBUF bias-copy runs on ACT (keeps DVE clear).
  phase 2: 512-step recurrence, per step per direction:
      ident-MM injects xp(t) into a full psum bank (start=True), then
      16 Whh matmuls (N=8, ~27ns issue floor each) accumulate gates;
      sigmoid[128,64] -> g2/tmp/fc/add (DVE, bf16) -> tanh -> h-mul
      (split by k-half so the next burst's kk=0 matmuls start early).
  h states staged 32 steps at a time, DMA'd to DRAM as bf16.

The step period is bound by the serial chain burst -> sigmoid -> c-update
-> tanh -> h-mul -> next burst (~2.35us per F+B step pair; ACT ops cost
~310ns fixed each, cross-engine semaphore hops ~60-200ns each).
"""

import sys

for _p in ("/opt/trn_rl_repo",):
    if _p not in sys.path:
        sys.path.insert(0, _p)

import numpy as np
import ml_dtypes

import concourse.bass as bass
import concourse.mybir as mybir
import concourse.tile as tile
from concourse.tile import add_dep_helper
from concourse import bacc
from concourse.bass_utils import run_bass_kernel_spmd

B, T, D, H = 64, 512, 256, 256
NCORES = 8
BC = B // NCORES          # 8 sequences per core
G4 = 4 * H                # 1024 gate dims
STG = 32                  # recurrence steps per output staging block

BF16 = mybir.dt.bfloat16
F32 = mybir.dt.float32
AF = mybir.ActivationFunctionType
# Whh dtype for the PE recurrence matmuls. fp8e3 with a x16 prescale (undone
# by sigmoid scale) also passes (rel err 6.2e-3) but gives no speedup: the
# burst is matmul-issue-floor bound (~27ns/MM), not weight-load bound.
WHH_DT = mybir.dt.bfloat16
WHH_NP = ml_dtypes.bfloat16
WHH_SCALE = 1.0

# gate order stays torch (i,f,g,o): o-gates last so sigmoid(i,f,g) can
# start before the o-gate matmuls finish.
_PERM = np.arange(4 * H)


def build_nc(t_steps=T):
    assert t_steps % STG == 0
    nb = t_steps // STG
    TB = t_steps * BC  # (t,b) columns per k-half of xT

    nc = bacc.Bacc(None, target_bir_lowering=False)

    xt_d, wih_d, whh_d, bias_d = {}, {}, {}, {}
    for d in ("f", "b"):
        xt_d[d] = nc.dram_tensor(f"xt_{d}", [128, 2 * TB], BF16, kind="ExternalInput")
        wih_d[d] = nc.dram_tensor(f"wih_{d}", [128, 2048], BF16, kind="ExternalInput")
        whh_d[d] = nc.dram_tensor(f"whh_{d}", [128, 2048], WHH_DT, kind="ExternalInput")
        bias_d[d] = nc.dram_tensor(f"bias_{d}", [128, 8], F32, kind="ExternalInput")
    ident_d = nc.dram_tensor("ident", [128, 128], BF16, kind="ExternalInput")
    out_e = nc.dram_tensor("out", [128, t_steps * 4 * BC], BF16, kind="ExternalOutput")

    with tile.TileContext(nc) as tc:
        with (
            tc.tile_pool(name="big", bufs=1) as big,
            tc.tile_pool(name="work", bufs=6) as work,
            tc.tile_pool(name="stgp", bufs=3) as stgp,
            tc.tile_pool(name="pp", bufs=2, space=bass.MemorySpace.PSUM) as pp,
            tc.tile_pool(name="pr", bufs=3, space=bass.MemorySpace.PSUM) as pr,
        ):
            xt, wih, whh, bias, xpt = {}, {}, {}, {}, {}
            for d in ("f", "b"):
                xt[d] = big.tile([128, 2 * TB], BF16, tag=f"xt{d}", name=f"xt{d}")
                nc.sync.dma_start(xt[d][:], xt_d[d][:])
                wih[d] = big.tile([128, 2048], BF16, tag=f"wih{d}", name=f"wih{d}")
                nc.sync.dma_start(wih[d][:], wih_d[d][:])
                whh[d] = big.tile([128, 2048], WHH_DT, tag=f"whh{d}", name=f"whh{d}")
                nc.sync.dma_start(whh[d][:], whh_d[d][:])
                bias[d] = big.tile([128, 8], F32, tag=f"bias{d}", name=f"bias{d}")
                nc.sync.dma_start(bias[d][:], bias_d[d][:])
                xpt[d] = big.tile([128, t_steps * 8 * BC], BF16, tag=f"xpt{d}", name=f"xpt{d}")
            zh = big.tile([128, 4 * BC], BF16, tag="zh", name="zh")
            nc.vector.memset(zh[:], 0.0)
            ident = big.tile([128, 128], BF16, tag="ident", name="ident")
            nc.sync.dma_start(ident[:], ident_d[:])

            # ---- phase 1: input projection (emitted lazily, paced into the
            # recurrence loop so it fills engine idle time instead of
            # blocking the first recurrence steps) ----
            # xpt layout per dir: col = j*TB + t*BC + b -> projection writes
            # are contiguous [128,512]; the recurrence I-MM reads a strided
            # [128, 8, BC] view.
            ncols = min(512, TB)
            ntiles = TB // ncols

            proj_groups = [
                (d, nt, j)
                for nt in range(ntiles)
                for d in ("f", "b")
                for j in range(8)
            ]

            def emit_proj_group(d, nt, j):
                ps = pp.tile([128, ncols], F32, tag="pp", name="pp")
                for kk in (0, 1):
                    nc.tensor.matmul(
                        ps[:],
                        wih[d][:, kk * 1024 + j * 128 : kk * 1024 + (j + 1) * 128],
                        xt[d][:, kk * TB + nt * ncols : kk * TB + (nt + 1) * ncols],
                        start=(kk == 0),
                        stop=(kk == 1),
                    )
                # psum -> SBUF copy with per-partition bias on ACT (keeps the
                # DVE FIFO free of 660ns blockers in front of the c-chain).
                nc.scalar.add(
                    xpt[d][:, j * TB + nt * ncols : j * TB + (nt + 1) * ncols],
                    ps[:], bias[d][:, j : j + 1],
                )

            # ---- phase 2: recurrence (staggered F/B emission) ----
            # psum(t) = I.T @ xp(t)  (start=True)  then += Whh tiles; the
            # sigmoid reads PSUM directly.  tanh(g) is folded into the wide
            # sigmoid: g rows were pre-scaled x2 on host, tanh(g)=2*sig(2g)-1.
            stg_tiles = {}

            def stg_slot(u):
                return stg_tiles[u // STG], (u % STG) * 4 * BC

            # ps tiles are full PSUM banks ([128,512] f32 = 2KB) so the
            # start=True bank-clear of one step's ident can never touch the
            # other chain's in-flight accumulation.
            def emit_ident(d, t):
                ps = pr.tile([128, 512], F32, tag=f"pr{d}", name=f"pr{d}")
                xv = xpt[d][:].rearrange("p (j tb) -> p j tb", j=8)
                mm = nc.tensor.matmul(
                    ps[:, : 8 * BC], ident[:], xv[:, :, t * BC : (t + 1) * BC],
                    start=True, stop=False,
                )
                return ps, mm

            def emit_whh(d, doff, t, ps):
                if t == 0:
                    prev = zh[:]
                else:
                    st, off = stg_slot(t - 1)
                    prev = st[:, off : off + 4 * BC]
                for kk in (0, 1):
                    rhs = prev[:, doff + kk * BC : doff + (kk + 1) * BC]
                    for j in range(8):
                        nc.tensor.matmul(
                            ps[:, j * BC : (j + 1) * BC],
                            whh[d][:, kk * 1024 + j * 128 : kk * 1024 + (j + 1) * 128],
                            rhs,
                            start=False,
                            stop=(j == 7 and kk == 1),
                        )

            # Elementwise pipeline runs in bf16: 2-byte operands enable the
            # DVE 2x perf mode (fp32 ADD-class DVE ops measure ~173ns vs ~79).
            EWT = BF16
            cst = {}
            last_hm0 = {}
            for d in ("f", "b"):
                c0 = work.tile([128, 2 * BC], EWT, tag=f"c{d}", name=f"c{d}", bufs=3)
                nc.vector.memset(c0[:], 0.0)
                cst[d] = c0

            def emit_ew(d, doff, t, ps):
                # gate layout (i,f,g,o): sigmoid of i,f,g starts after 12 of
                # the 16 Whh matmuls; sigmoid(o) only gates the h-mul tail.
                st, off = stg_slot(t)
                act = work.tile([128, 8 * BC], EWT, tag=f"act{d}", name=f"act{d}")
                nc.scalar.activation(
                    act[:], ps[:, : 8 * BC], AF.Sigmoid, scale=1.0 / WHH_SCALE
                )
                # g2 = tanh(g) = 2*sig(2g)-1 (g rows pre-scaled x2 on host)
                g2 = work.tile([128, 2 * BC], EWT, tag=f"g2{d}", name=f"g2{d}")
                nc.vector.tensor_scalar(
                    g2[:], act[:, 4 * BC : 6 * BC], -0.5, 2.0,
                    mybir.AluOpType.add, mybir.AluOpType.mult,
                )
                tmp = work.tile([128, 2 * BC], EWT, tag=f"tmp{d}", name=f"tmp{d}")
                nc.vector.tensor_mul(tmp[:], act[:, : 2 * BC], g2[:])
                fc = work.tile([128, 2 * BC], EWT, tag=f"fc{d}", name=f"fc{d}")
                nc.vector.tensor_mul(fc[:], act[:, 2 * BC : 4 * BC], cst[d][:])
                cnew = work.tile([128, 2 * BC], EWT, tag=f"c{d}", name=f"c{d}", bufs=3)
                add_ins = nc.vector.tensor_add(cnew[:], fc[:], tmp[:])
                other = "b" if d == "f" else "f"
                if last_hm0.get(other) is not None:
                    add_dep_helper(add_ins.ins, last_hm0[other].ins, sync=False,
                                   reason="keep h-mul ahead of other chain's c-add")
                cst[d] = cnew
                th = work.tile([128, 2 * BC], EWT, tag=f"th{d}", name=f"th{d}")
                tanh_ins = nc.scalar.activation(th[:], cnew[:], AF.Tanh)
                # h-mul split by k-half: the next burst's kk=0 matmuls only
                # need h[:, :BC], so they can start while the second half of
                # the h-mul still runs.
                hm0 = nc.vector.tensor_mul(
                    st[:, off + doff : off + doff + BC],
                    act[:, 6 * BC : 7 * BC],
                    th[:, :BC],
                )
                nc.vector.tensor_mul(
                    st[:, off + doff + BC : off + doff + 2 * BC],
                    act[:, 7 * BC : 8 * BC],
                    th[:, BC:],
                )
                last_hm0[d] = hm0
                return tanh_ins

            # upfront: the first two ntiles (steps 0..127); the rest paced
            gq = list(proj_groups)
            n_upfront = min(len(gq), 32)
            for _ in range(n_upfront):
                emit_proj_group(*gq.pop(0))
            n_rest = len(gq)

            stg_tiles[0] = stgp.tile([128, STG * 4 * BC], BF16, tag="stg", name="stg")
            ps_f, _ = emit_ident("f", 0)
            ps_b, _ = emit_ident("b", 0)
            ps_f_next = ps_b_next = None
            for t in range(t_steps):
                if n_rest:
                    tgt = min(n_rest, (t * n_rest) // max(1, (t_steps - 128)) + 1)
                    while len(gq) > n_rest - tgt:
                        emit_proj_group(*gq.pop(0))
                if t % STG == 0 and t > 0:
                    stg_tiles[t // STG] = stgp.tile(
                        [128, STG * 4 * BC], BF16, tag="stg", name="stg"
                    )
                emit_whh("f", 0, t, ps_f)
                if t + 1 < t_steps:
                    ps_f_next, _ = emit_ident("f", t + 1)
                if t >= 1:
                    emit_ew("b", 2 * BC, t - 1, ps_b_prev)
                    if t % STG == 0:
                        blk = t // STG - 1
                        nc.sync.dma_start(
                            out_e[:, blk * STG * 4 * BC : (blk + 1) * STG * 4 * BC],
                            stg_tiles[blk][:],
                        )
                emit_whh("b", 2 * BC, t, ps_b)
                if t + 1 < t_steps:
                    ps_b_next, identb_mm = emit_ident("b", t + 1)
                emit_ew("f", 0, t, ps_f)
                ps_b_prev = ps_b
                ps_f, ps_b = ps_f_next, ps_b_next
            emit_ew("b", 2 * BC, t_steps - 1, ps_b_prev)
            blk = nb - 1
            nc.sync.dma_start(
                out_e[:, blk * STG * 4 * BC : (blk + 1) * STG * 4 * BC],
                stg_tiles[blk][:],
            )

    nc.compile()
    return nc


def _prep_core(xs, Wih, Whh, bih, bhh, t_steps):
    """Host-side layout prep for one core, one direction.

    xs: [BC, t, D] f32 (already reversed for the backward direction).
    Returns dict of device arrays.
    """
    TB = t_steps * BC
    Wp = Wih[_PERM].astype(np.float32).copy()   # [1024, 256]
    Wh = Whh[_PERM].astype(np.float32).copy()
    bsum = (bih + bhh)[_PERM].astype(np.float32).copy()
    # tanh(g) is computed as 2*sigmoid(2g)-1 on device: pre-scale g rows x2
    Wp[2 * H : 3 * H] *= 2.0
    Wh[2 * H : 3 * H] *= 2.0
    bsum[2 * H : 3 * H] *= 2.0
    # everything feeding psum is prescaled x16 (undone by sigmoid scale) so
    # the fp8e3 Whh lands in e3m4's normal range
    Wp *= WHH_SCALE
    Wh *= WHH_SCALE
    bsum *= WHH_SCALE

    def wt_layout(W, dtype=ml_dtypes.bfloat16):  # [4H, 256] -> [128, 2048] lhsT
        WT = W.T.reshape(2, 128, G4).transpose(1, 0, 2).reshape(128, 2 * G4)
        return np.ascontiguousarray(WT).astype(dtype)

    xT = (
        xs.transpose(2, 1, 0)                   # [256, t, BC]
        .reshape(2, 128, TB)
        .transpose(1, 0, 2)
        .reshape(128, 2 * TB)
    )
    return {
        "xt": np.ascontiguousarray(xT).astype(ml_dtypes.bfloat16),
        "wih": wt_layout(Wp),
        "whh": wt_layout(Wh, WHH_NP),
        "bias": np.ascontiguousarray(bsum.reshape(8, 128).T).astype(np.float32),
    }


_NC_CACHE = {}


def _get_nc(t_steps):
    if t_steps not in _NC_CACHE:
        _NC_CACHE[t_steps] = build_nc(t_steps)
    return _NC_CACHE[t_steps]


def kernel(x, input_length, Wih_f, Whh_f, bih_f, bhh_f, Wih_b, Whh_b, bih_b, bhh_b,
           t_steps=T, _want_trace=False):
    x = np.asarray(x, np.float32)
    lens = np.asarray(input_length).astype(np.int64)
    L = t_steps
    tt = np.arange(L)

    nc = _get_nc(t_steps)

    in_maps = []
    for c in range(NCORES):
        bs = slice(c * BC, (c + 1) * BC)
        xs = x[bs, :L]
        ls = lens[bs]
        inv_idx = L - 1 - ((L - ls[:, None] + tt[None, :]) % L)       # [BC, L]
        xn = np.take_along_axis(xs, inv_idx[:, :, None], axis=1)
        pf = _prep_core(xs, Wih_f, Whh_f, bih_f, bhh_f, L)
        pb = _prep_core(xn, Wih_b, Whh_b, bih_b, bhh_b, L)
        in_maps.append(
            {
                "xt_f": pf["xt"], "wih_f": pf["wih"], "whh_f": pf["whh"], "bias_f": pf["bias"],
                "xt_b": pb["xt"], "wih_b": pb["wih"], "whh_b": pb["whh"], "bias_b": pb["bias"],
                "ident": np.eye(128, dtype=np.float32).astype(ml_dtypes.bfloat16),
            }
        )

    kw = {}
    if _want_trace:
        kw = dict(trace=True)
    res = run_bass_kernel_spmd(nc, in_maps, core_ids=list(range(NCORES)), **kw)

    outs = []
    for c in range(NCORES):
        bs = slice(c * BC, (c + 1) * BC)
        ls = lens[bs]
        arr = np.asarray(res.results[c]["out"]).astype(np.float32)
        arr = arr.reshape(128, L, 4, BC)
        fwd = arr[:, :, 0:2, :].transpose(3, 1, 2, 0).reshape(BC, L, 2 * 128)
        bwd = arr[:, :, 2:4, :].transpose(3, 1, 2, 0).reshape(BC, L, 2 * 128)
        bwd_idx = np.clip(ls[:, None] - 1 - tt[None, :], 0, L - 1)
        bwd_g = np.take_along_axis(bwd, bwd_idx[:, :, None], axis=1)
        o = np.concatenate([fwd, bwd_g], axis=-1)
        mask = (tt[None, :] < ls[:, None])[:, :, None]
        outs.append(np.where(mask, o, 0.0).astype(np.float32))
    full = np.concatenate(outs, axis=0)
    if _want_trace:
        return full, res
    return full

